# revision 1
# baseline (speedup 1.0000x reference)
"""Trainium2 Bass kernel for BinaryTokenClassificationModel (segment_reduce).

Reference semantics (B=16, L=2048, H=1024, W=1024):
    src = segment_mean(hidden, source_word_ids)   # [B,W,H]
    tgt = segment_mean(hidden, target_word_ids)   # [B,W,H]
    logits[b,s,t,0] = src[b,s]@w_s + tgt[b,t]@w_t + bias

Key algebraic restructuring: the pooled [B,W,H] tensors are never
materialized.  Because the classifier is linear,
    src_proj[b,s] = segment_mean_s( hidden[b,l] @ w_s )
so we compute per-token scalar dots (fused multiply+reduce on the DVE),
segment-reduce the *scalars* (via tiny one-hot matmuls on the PE, using
the factorization w = 128*q + r), and emit the [W,W] output as an outer
broadcast-sum.

Sharding: data-parallel over batch — 2 examples per NeuronCore on 8 cores.
The tiny classifier weights are replicated.
"""

from contextlib import ExitStack

import ml_dtypes
import numpy as np

import concourse.mybir as mybir
import concourse.tile as tile
from concourse import bacc
from concourse.bass_utils import run_bass_kernel_spmd
from concourse.masks import make_identity

P = 128          # partitions
B = 16           # full batch
NCORES = 8
BLOC = B // NCORES   # batches per core = 2
L = 2048         # tokens
H = 1024         # hidden
W = 1024         # words
Q = W // P       # 8 word chunks
NI = L // P      # 16 token tiles per batch (token l = p*NI + i)
ICH = 4          # token tiles loaded per DMA

F32 = mybir.dt.float32
BF16 = mybir.dt.bfloat16
I32 = mybir.dt.int32

# Compute dtype for the per-token dot products. "bf16" halves both the
# hidden-states DMA traffic and the DVE multiply cost (2x packed mode);
# reductions stay fp32 (DVE/ACT accumulate internally in fp32).
DOT_DTYPE = "bf16"
HDT = BF16 if DOT_DTYPE == "bf16" else F32
# Fraction of dot tiles reduced via the fused DVE op (affine_mul_reduce);
# the rest go DVE-mul + ACT-accumulate to balance engine load.
AMR_PATTERN = 8   # (tile_index % AMR_PATTERN) < AMR_KEEP -> fused DVE path
AMR_KEEP = 3

_CACHE = {}


def _build_module():
    nc = bacc.Bacc(None, target_bir_lowering=False, debug=False)
    names = {}
    with tile.TileContext(nc) as tc, ExitStack() as ctx:
        dram = ctx.enter_context(tc.tile_pool(name="dram", bufs=1, space="DRAM"))
        sb_c = ctx.enter_context(tc.tile_pool(name="const", bufs=1))
        sb_h = ctx.enter_context(tc.tile_pool(name="hid", bufs=4))
        sb_s = ctx.enter_context(tc.tile_pool(name="small", bufs=2))
        sb_o = ctx.enter_context(tc.tile_pool(name="outp", bufs=6))
        ps = ctx.enter_context(tc.tile_pool(name="psum", bufs=2, space="PSUM"))

        hid_d = [dram.tile([L, H], HDT, kind="ExternalInput", name=f"hid{b}")
                 for b in range(BLOC)]
        src_d = [dram.tile([L], I32, kind="ExternalInput", name=f"srcids{b}")
                 for b in range(BLOC)]
        tgt_d = [dram.tile([L], I32, kind="ExternalInput", name=f"tgtids{b}")
                 for b in range(BLOC)]
        w_d = dram.tile([P, 2 * H], HDT, kind="ExternalInput")
        b_d = dram.tile([P, 1], F32, kind="ExternalInput")
        out_d = [dram.tile([W, W], F32, kind="ExternalOutput", name=f"logits{b}")
                 for b in range(BLOC)]

        names["hid"] = [t.name for t in hid_d]
        names["src"] = [t.name for t in src_d]
        names["tgt"] = [t.name for t in tgt_d]
        names["w"] = w_d.name
        names["b"] = b_d.name
        names["out"] = [t.name for t in out_d]

        # ---- constants ----
        w_s = sb_c.tile([P, H], HDT, tag="ws")
        w_t = sb_c.tile([P, H], HDT, tag="wt")
        nc.scalar.dma_start(out=w_s[:], in_=w_d[:, 0:H])
        nc.scalar.dma_start(out=w_t[:], in_=w_d[:, H:2 * H])
        b_bc = sb_c.tile([P, 1], F32, tag="bb")
        nc.scalar.dma_start(out=b_bc[:], in_=b_d[:])

        # iota_r16[p, i, r] = r ; iota_q16[p, i, q] = q  (batched one-hot builds)
        iota_r16 = sb_c.tile([P, NI, P], F32, tag="ior")
        nc.gpsimd.iota(iota_r16[:], pattern=[[0, NI], [1, P]], base=0,
                       channel_multiplier=0, allow_small_or_imprecise_dtypes=True)
        iota_q16 = sb_c.tile([P, NI, Q], F32, tag="ioq")
        nc.gpsimd.iota(iota_q16[:], pattern=[[0, NI], [1, Q]], base=0,
                       channel_multiplier=0, allow_small_or_imprecise_dtypes=True)
        ident = sb_c.tile([P, P], F32, tag="id")
        make_identity(nc, ident[:])
        ones = sb_c.tile([P, P], F32, tag="ones")
        nc.vector.memset(ones[:], 1.0)

        for b in range(BLOC):
            hid_ap = hid_d[b][:].rearrange("(p i) h -> p i h", p=P)  # l = p*NI+i

            # ---- ids -> (q, r) one-hots, before the dot loop ----
            qf = {}
            rf = {}
            for side, ids_dram in (("s", src_d[b]), ("t", tgt_d[b])):
                ids_t = sb_s.tile([P, NI], I32, tag="ids")
                nc.sync.dma_start(out=ids_t[:],
                                  in_=ids_dram[:].rearrange("(p i) -> p i", p=P))
                q_i = sb_s.tile([P, NI], I32, tag="qi")
                r_i = sb_s.tile([P, NI], I32, tag="ri")
                nc.vector.tensor_scalar(out=q_i[:], in0=ids_t[:], scalar1=7,
                                        scalar2=None,
                                        op0=mybir.AluOpType.logical_shift_right)
                nc.vector.tensor_scalar(out=r_i[:], in0=ids_t[:], scalar1=127,
                                        scalar2=None,
                                        op0=mybir.AluOpType.bitwise_and)
                qf[side] = sb_s.tile([P, NI], F32, tag=f"qf{side}", name=f"qf{side}")
                rf[side] = sb_s.tile([P, NI], F32, tag=f"rf{side}", name=f"rf{side}")
                nc.vector.tensor_copy(out=qf[side][:], in_=q_i[:])
                nc.vector.tensor_copy(out=rf[side][:], in_=r_i[:])

            or_all = {}
            mdoq = {}
            segT = {}
            dots = {}
            for side in ("s", "t"):
                or_all[side] = sb_s.tile([P, NI, P], F32, tag=f"orall{side}",
                                         name=f"orall{side}")
                nc.vector.tensor_tensor(
                    out=or_all[side][:], in0=iota_r16[:],
                    in1=rf[side][:].to_broadcast([P, NI, P]),
                    op=mybir.AluOpType.is_equal)
                mdoq[side] = sb_s.tile([P, NI, 2 * Q], F32, tag=f"mdoq{side}",
                                       name=f"mdoq{side}")
                nc.vector.tensor_tensor(
                    out=mdoq[side][:, :, Q:2 * Q], in0=iota_q16[:],
                    in1=qf[side][:].to_broadcast([P, NI, Q]),
                    op=mybir.AluOpType.is_equal)
                segT[side] = ps.tile([2 * Q, P], F32, space="PSUM",
                                     tag=f"segT{side}", name=f"segT{side}")
                dots[side] = sb_s.tile([P, NI], F32, tag=f"dots{side}",
                                       name=f"dots{side}")

            # ---- dots (fused mul+reduce on DVE) + interleaved seg matmuls ----
            for ic in range(NI // ICH):
                ht = sb_h.tile([P, ICH, H], HDT, tag="ht")
                nc.sync.dma_start(out=ht[:], in_=hid_ap[:, ic * ICH:(ic + 1) * ICH, :])
                for k in range(ICH):
                    i = ic * ICH + k
                    for sidx, (side, wt) in enumerate((("t", w_t), ("s", w_s))):
                        scratch = sb_s.tile([P, H], HDT, tag="scr", bufs=6)
                        if HDT is F32 or (2 * i + sidx) % AMR_PATTERN < AMR_KEEP:
                            nc.vector.affine_mul_reduce(
                                out=scratch[:], accum_out=dots[side][:, i:i + 1],
                                in0=ht[:, k, :], in1=wt[:], scale=1.0, bias=0.0)
                        else:
                            nc.vector.tensor_tensor(
                                out=scratch[:], in0=ht[:, k, :], in1=wt[:],
                                op=mybir.AluOpType.mult)
                            scratch2 = sb_s.tile([P, H], HDT, tag="scr2", bufs=6)
                            nc.scalar.activation(
                                out=scratch2[:], in_=scratch[:],
                                func=mybir.ActivationFunctionType.Copy,
                                accum_out=dots[side][:, i:i + 1])
                # this chunk's md columns, then its segment matmuls
                sl = slice(ic * ICH, (ic + 1) * ICH)
                for side in ("s", "t"):
                    nc.vector.tensor_tensor(
                        out=mdoq[side][:, sl, 0:Q], in0=mdoq[side][:, sl, Q:2 * Q],
                        in1=dots[side][:, sl].to_broadcast([P, ICH, Q]),
                        op=mybir.AluOpType.mult)
                    for k in range(ICH):
                        i = ic * ICH + k
                        nc.tensor.matmul(out=segT[side][:],
                                         lhsT=mdoq[side][:, i, :],
                                         rhs=or_all[side][:, i, :],
                                         start=(i == 0), stop=(i == NI - 1))

            # ---- per-side epilogue: transpose back, divide ----
            proj = {}
            for side in ("t", "s"):
                segT_sb = sb_s.tile([2 * Q, P], F32, tag="segTsb", name="segTsb")
                nc.scalar.copy(out=segT_sb[:], in_=segT[side][:])
                seg_ps = ps.tile([P, 2 * Q], F32, space="PSUM", tag="seg",
                                 name="seg")
                nc.tensor.transpose(out=seg_ps[:], in_=segT_sb[:],
                                    identity=ident[0:2 * Q, 0:2 * Q])
                cnt = sb_s.tile([P, Q], F32, tag="cnt")
                nc.vector.tensor_scalar(out=cnt[:], in0=seg_ps[:, Q:2 * Q],
                                        scalar1=1.0, scalar2=None,
                                        op0=mybir.AluOpType.max)
                rec = sb_s.tile([P, Q], F32, tag="rec")
                nc.vector.reciprocal(out=rec[:], in_=cnt[:])
                proj[side] = sb_s.tile([P, Q], F32, tag=f"proj{side}", name=f"proj{side}")
                nc.vector.tensor_tensor(out=proj[side][:], in0=seg_ps[:, 0:Q],
                                        in1=rec[:], op=mybir.AluOpType.mult)

            # fold bias into source projection
            proj_sb = sb_s.tile([P, Q], F32, tag="projsb")
            nc.vector.tensor_scalar(out=proj_sb[:], in0=proj["s"][:],
                                    scalar1=b_bc[:, 0:1], scalar2=None,
                                    op0=mybir.AluOpType.add)

            # ---- broadcast tgt projection to a [P, W] row: tp[p, q*128+r] = proj_t[r, q]
            msel = sb_s.tile([P, W], F32, tag="msel")
            for qb in range(Q):
                nc.vector.tensor_scalar(
                    out=msel[:, qb * P:(qb + 1) * P], in0=ident[:],
                    scalar1=proj["t"][:, qb:qb + 1], scalar2=None,
                    op0=mybir.AluOpType.mult)
            bc_sb = sb_s.tile([P, W], F32, tag="bcsb")
            for half in range(2):
                bc_ps = ps.tile([P, W // 2], F32, space="PSUM", tag="bc")
                nc.tensor.matmul(out=bc_ps[:], lhsT=ones[:],
                                 rhs=msel[:, half * (W // 2):(half + 1) * (W // 2)],
                                 start=True, stop=True)
                nc.scalar.copy(out=bc_sb[:, half * (W // 2):(half + 1) * (W // 2)],
                               in_=bc_ps[:])

            # ---- output tiles: out[j*128+p, t] = proj_s[p, j] + tp[t] ----
            out_ap = out_d[b][:].rearrange("(j p) t -> p j t", p=P)
            for j in range(Q):
                ot = sb_o.tile([P, W], F32, tag="ot")
                if b == BLOC - 1 and j % 2 == 0:
                    # tail batch: split adds across DVE and ACT
                    nc.vector.tensor_scalar(
                        out=ot[:], in0=bc_sb[:], scalar1=proj_sb[:, j:j + 1],
                        scalar2=None, op0=mybir.AluOpType.add)
                else:
                    nc.scalar.add(out=ot[:], in_=bc_sb[:], add=proj_sb[:, j:j + 1])
                nc.scalar.dma_start(out=out_ap[:, j, :], in_=ot[:])

    nc.compile()
    return nc, names


def _get_module():
    if "mod" not in _CACHE:
        _CACHE["mod"] = _build_module()
    return _CACHE["mod"]


def _run(hidden, classifier_w, classifier_b, source_word_ids, target_word_ids,
         **spmd_kwargs):
    nc, names = _get_module()
    hdtype = ml_dtypes.bfloat16 if DOT_DTYPE == "bf16" else np.float32
    hidden = np.ascontiguousarray(hidden).astype(hdtype, copy=False)
    w = np.ascontiguousarray(
        np.broadcast_to(np.asarray(classifier_w, dtype=np.float32)
                        .reshape(1, 2 * H), (P, 2 * H)).astype(hdtype))
    bias = np.ascontiguousarray(
        np.broadcast_to(np.asarray(classifier_b, dtype=np.float32)
                        .reshape(1, 1), (P, 1)))
    src = np.ascontiguousarray(source_word_ids, dtype=np.int32)
    tgt = np.ascontiguousarray(target_word_ids, dtype=np.int32)

    in_maps = []
    for c in range(NCORES):
        m = {names["w"]: w, names["b"]: bias}
        for b in range(BLOC):
            gb = c * BLOC + b
            m[names["hid"][b]] = hidden[gb]
            m[names["src"][b]] = src[gb]
            m[names["tgt"][b]] = tgt[gb]
        in_maps.append(m)

    res = run_bass_kernel_spmd(nc, in_maps, core_ids=list(range(NCORES)),
                               **spmd_kwargs)
    out = np.empty((B, W, W, 1), dtype=np.float32)
    for c in range(NCORES):
        for b in range(BLOC):
            out[c * BLOC + b, :, :, 0] = res.results[c][names["out"][b]]
    return out, res


def kernel(hidden, classifier_w, classifier_b, source_word_ids,
           target_word_ids, num_words):
    out, _ = _run(hidden, classifier_w, classifier_b, source_word_ids,
                  target_word_ids)
    return out



# revision 5
# speedup vs baseline: 1.4386x; 1.4386x over previous
"""Trainium2 Bass kernel for BinaryTokenClassificationModel (segment_reduce).

Reference semantics (B=16, L=2048, H=1024, W=1024):
    src = segment_mean(hidden, source_word_ids)   # [B,W,H]
    tgt = segment_mean(hidden, target_word_ids)   # [B,W,H]
    logits[b,s,t,0] = src[b,s]@w_s + tgt[b,t]@w_t + bias

Because the classifier is linear, the pooled [B,W,H] tensors are never
materialized:
    src_proj[b,s] = segment_mean_s( hidden[b,l] @ w_s )
so the per-token scalar dots are computed on the PE (hidden is
pre-transposed to [H,L] on the host so it streams through the PE as the
moving operand against a tiny stationary [128,2] weight tile), the
scalar dots are segment-reduced via one-hot matmuls on the PE (word =
128*q + r factorization), and the [W,W] output is an outer
broadcast-sum emitted in bf16 (upcast on the host).

Sharding: data-parallel over batch — 2 examples per NeuronCore on 8
cores; classifier weights replicated.
"""

from contextlib import ExitStack

import ml_dtypes
import numpy as np

import concourse.mybir as mybir
import concourse.tile as tile
from concourse import bacc
from concourse.bass_utils import run_bass_kernel_spmd
from concourse.masks import make_identity

P = 128          # partitions
B = 16           # full batch
NCORES = 8
BLOC = B // NCORES   # batches per core = 2
L = 2048         # tokens
H = 1024         # hidden
W = 1024         # words
Q = W // P       # 8 word chunks
NI = L // P      # 16 token tiles per batch (token l = p*NI + i)
NH = H // P      # 8 hidden chunks
NCH = 4          # 512-token psum chunks for the dot matmuls
CHW = L // NCH   # 512

F32 = mybir.dt.float32
BF16 = mybir.dt.bfloat16
I32 = mybir.dt.int32

_CACHE = {}


def _build_module():
    nc = bacc.Bacc(None, target_bir_lowering=False, debug=False)
    names = {}
    with tile.TileContext(nc) as tc, ExitStack() as ctx:
        dram = ctx.enter_context(tc.tile_pool(name="dram", bufs=1, space="DRAM"))
        sb_c = ctx.enter_context(tc.tile_pool(name="const", bufs=1))
        sb_h = ctx.enter_context(tc.tile_pool(name="hid", bufs=3))
        sb_s = ctx.enter_context(tc.tile_pool(name="small", bufs=2))
        sb_o = ctx.enter_context(tc.tile_pool(name="outp", bufs=4))
        ps_d = ctx.enter_context(tc.tile_pool(name="psdot", bufs=1, space="PSUM"))
        ps = ctx.enter_context(tc.tile_pool(name="psum", bufs=1, space="PSUM"))

        hidT_d = [dram.tile([H, L], BF16, kind="ExternalInput", name=f"hidT{b}")
                  for b in range(BLOC)]
        src_d = [dram.tile([L], I32, kind="ExternalInput", name=f"srcids{b}")
                 for b in range(BLOC)]
        tgt_d = [dram.tile([L], I32, kind="ExternalInput", name=f"tgtids{b}")
                 for b in range(BLOC)]
        w_d = dram.tile([H, 2], BF16, kind="ExternalInput")   # [:,0]=w_s [:,1]=w_t
        b_d = dram.tile([P, 1], F32, kind="ExternalInput")
        out_d = [dram.tile([W, W], BF16, kind="ExternalOutput", name=f"logits{b}")
                 for b in range(BLOC)]

        names["hidT"] = [t.name for t in hidT_d]
        names["src"] = [t.name for t in src_d]
        names["tgt"] = [t.name for t in tgt_d]
        names["w"] = w_d.name
        names["b"] = b_d.name
        names["out"] = [t.name for t in out_d]

        # ---- constants ----
        w2 = sb_c.tile([P, NH, 2], BF16, tag="w2")
        nc.scalar.dma_start(out=w2[:], in_=w_d[:].rearrange("(hq hp) s -> hp hq s",
                                                            hp=P))
        b_bc = sb_c.tile([P, 1], F32, tag="bb")
        nc.scalar.dma_start(out=b_bc[:], in_=b_d[:])

        # iota_r16[p, i, r] = r ; iota_q16[p, i, q] = q  (batched one-hot builds)
        iota_r16 = sb_c.tile([P, NI, P], F32, tag="ior")
        nc.gpsimd.iota(iota_r16[:], pattern=[[0, NI], [1, P]], base=0,
                       channel_multiplier=0, allow_small_or_imprecise_dtypes=True)
        iota_q16 = sb_c.tile([P, NI, Q], F32, tag="ioq")
        nc.gpsimd.iota(iota_q16[:], pattern=[[0, NI], [1, Q]], base=0,
                       channel_multiplier=0, allow_small_or_imprecise_dtypes=True)
        ident = sb_c.tile([P, P], BF16, tag="id")
        make_identity(nc, ident[:])
        ones = sb_c.tile([P, P], BF16, tag="ones")
        nc.vector.memset(ones[:], 1.0)

        for b in range(BLOC):
            # ---- ids -> (q, r) one-hots (DVE), independent of the dots ----
            oralls = {}
            mdoq = {}
            for side, ids_dram in (("s", src_d[b]), ("t", tgt_d[b])):
                ids_t = sb_s.tile([P, NI], I32, tag="ids")
                nc.sync.dma_start(out=ids_t[:],
                                  in_=ids_dram[:].rearrange("(p i) -> p i", p=P))
                q_i = sb_s.tile([P, NI], I32, tag="qi")
                r_i = sb_s.tile([P, NI], I32, tag="ri")
                nc.vector.tensor_scalar(out=q_i[:], in0=ids_t[:], scalar1=7,
                                        scalar2=None,
                                        op0=mybir.AluOpType.logical_shift_right)
                nc.vector.tensor_scalar(out=r_i[:], in0=ids_t[:], scalar1=127,
                                        scalar2=None,
                                        op0=mybir.AluOpType.bitwise_and)
                qf = sb_s.tile([P, NI], F32, tag="qf")
                rf = sb_s.tile([P, NI], F32, tag="rf")
                nc.vector.tensor_copy(out=qf[:], in_=q_i[:])
                nc.vector.tensor_copy(out=rf[:], in_=r_i[:])
                oralls[side] = sb_s.tile([P, NI, P], BF16, tag=f"orall{side}",
                                         name=f"orall{side}")
                nc.vector.tensor_tensor(
                    out=oralls[side][:], in0=iota_r16[:],
                    in1=rf[:].to_broadcast([P, NI, P]),
                    op=mybir.AluOpType.is_equal)
                mdoq[side] = sb_s.tile([P, NI, 2 * Q], BF16, tag=f"mdoq{side}",
                                       name=f"mdoq{side}")
                nc.vector.tensor_tensor(
                    out=mdoq[side][:, :, Q:2 * Q], in0=iota_q16[:],
                    in1=qf[:].to_broadcast([P, NI, Q]),
                    op=mybir.AluOpType.is_equal)

            # ---- per-token dots on the PE ----
            # psum chunk n holds dots for tokens [n*512, (n+1)*512) as
            # [2 sides, 512]; token l = p*NI + i indexes free dim directly.
            pdots = [ps_d.tile([2, P // NCH, NI], F32, space="PSUM", tag=f"pd{n}",
                               name=f"pd{n}") for n in range(NCH)]
            for hq in range(NH):
                ht = sb_h.tile([P, L], BF16, tag="ht")
                nc.sync.dma_start(out=ht[:],
                                  in_=hidT_d[b][hq * P:(hq + 1) * P, :])
                for n in range(NCH):
                    nc.tensor.matmul(out=pdots[n][:], lhsT=w2[:, hq, :],
                                     rhs=ht[:, n * CHW:(n + 1) * CHW],
                                     start=(hq == 0), stop=(hq == NH - 1))

            # dots_sb[s, p, i] = dot of token l = p*NI + i with w_side[s]
            dots_sb = sb_s.tile([2, P, NI], BF16, tag="dots", name="dots")
            for n in range(NCH):
                nc.scalar.copy(
                    out=dots_sb[:, n * (P // NCH):(n + 1) * (P // NCH), :],
                    in_=pdots[n][:])

            # transpose to token-partition layout: dotsT[p, i, s]
            psT = ps.tile([P, NI, 2], BF16, space="PSUM", tag="psT", name="psT")
            for i in range(NI):
                nc.tensor.transpose(out=psT[:, i, :], in_=dots_sb[:, :, i],
                                    identity=ident[0:2, 0:2])
            dotsT = sb_s.tile([P, NI, 2], BF16, tag="dotsT", name="dotsT")
            nc.scalar.copy(out=dotsT[:], in_=psT[:])

            # ---- md = onehot_q * dots, then segment matmuls on the PE ----
            # segP[r, side, col]: col 0:Q = per-q dot sums, Q:2Q = counts.
            segP = ps.tile([P, 2, 2 * Q], F32, space="PSUM", tag="segP",
                           name="segP")
            for sidx, side in enumerate(("s", "t")):
                nc.vector.tensor_tensor(
                    out=mdoq[side][:, :, 0:Q], in0=mdoq[side][:, :, Q:2 * Q],
                    in1=dotsT[:, :, sidx:sidx + 1].to_broadcast([P, NI, Q]),
                    op=mybir.AluOpType.mult)
                for i in range(NI):
                    nc.tensor.matmul(out=segP[:, sidx, :],
                                     lhsT=oralls[side][:, i, :],
                                     rhs=mdoq[side][:, i, :],
                                     start=(i == 0), stop=(i == NI - 1))

            # ---- divide sums by counts ----
            proj = {}
            for sidx, side in enumerate(("s", "t")):
                cnt = sb_s.tile([P, Q], F32, tag="cnt")
                nc.vector.tensor_scalar(out=cnt[:], in0=segP[:, sidx, Q:2 * Q],
                                        scalar1=1.0, scalar2=None,
                                        op0=mybir.AluOpType.max)
                rec = sb_s.tile([P, Q], F32, tag="rec")
                nc.vector.reciprocal(out=rec[:], in_=cnt[:])
                proj[side] = sb_s.tile([P, Q], F32, tag=f"proj{side}",
                                       name=f"proj{side}")
                nc.vector.tensor_tensor(out=proj[side][:],
                                        in0=segP[:, sidx, 0:Q],
                                        in1=rec[:], op=mybir.AluOpType.mult)

            # fold bias into source projection
            proj_sb = sb_s.tile([P, Q], F32, tag="projsb")
            nc.vector.tensor_scalar(out=proj_sb[:], in0=proj["s"][:],
                                    scalar1=b_bc[:, 0:1], scalar2=None,
                                    op0=mybir.AluOpType.add)

            # ---- broadcast tgt projection to a [P, W] row:
            #      tp[p, q*128+r] = proj_t[r, q]
            msel = sb_s.tile([P, W], BF16, tag="msel")
            for qb in range(Q):
                nc.vector.tensor_scalar(
                    out=msel[:, qb * P:(qb + 1) * P], in0=ident[:],
                    scalar1=proj["t"][:, qb:qb + 1], scalar2=None,
                    op0=mybir.AluOpType.mult)
            bc_sb = sb_s.tile([P, W], BF16, tag="bcsb")
            for half in range(2):
                bc_ps = ps.tile([P, W // 2], F32, space="PSUM", tag="bc")
                nc.tensor.matmul(out=bc_ps[:], lhsT=ones[:],
                                 rhs=msel[:, half * (W // 2):(half + 1) * (W // 2)],
                                 start=True, stop=True)
                nc.scalar.copy(out=bc_sb[:, half * (W // 2):(half + 1) * (W // 2)],
                               in_=bc_ps[:])

            # ---- output tiles: out[j*128+p, t] = proj_s[p, j] + tp[t] ----
            out_ap = out_d[b][:].rearrange("(j p) t -> p j t", p=P)
            for j in range(Q):
                ot = sb_o.tile([P, W], BF16, tag="ot")
                if j % 2 == 0:
                    nc.vector.tensor_scalar(
                        out=ot[:], in0=bc_sb[:], scalar1=proj_sb[:, j:j + 1],
                        scalar2=None, op0=mybir.AluOpType.add)
                else:
                    nc.scalar.add(out=ot[:], in_=bc_sb[:], add=proj_sb[:, j:j + 1])
                nc.sync.dma_start(out=out_ap[:, j, :], in_=ot[:])

    nc.compile()
    return nc, names


def _get_module():
    if "mod" not in _CACHE:
        _CACHE["mod"] = _build_module()
    return _CACHE["mod"]


def _run(hidden, classifier_w, classifier_b, source_word_ids, target_word_ids,
         **spmd_kwargs):
    nc, names = _get_module()
    bf16 = ml_dtypes.bfloat16
    # [B, H, L] contiguous so each [128, L] h-chunk is a dense DMA
    hidT = np.ascontiguousarray(
        np.asarray(hidden).astype(bf16).transpose(0, 2, 1))
    w = np.asarray(classifier_w, dtype=np.float32).reshape(2 * H)
    w2 = np.ascontiguousarray(np.stack([w[:H], w[H:]], axis=-1).astype(bf16))
    bias = np.ascontiguousarray(
        np.broadcast_to(np.asarray(classifier_b, dtype=np.float32)
                        .reshape(1, 1), (P, 1)))
    src = np.ascontiguousarray(source_word_ids, dtype=np.int32)
    tgt = np.ascontiguousarray(target_word_ids, dtype=np.int32)

    in_maps = []
    for c in range(NCORES):
        m = {names["w"]: w2, names["b"]: bias}
        for b in range(BLOC):
            gb = c * BLOC + b
            m[names["hidT"][b]] = hidT[gb]
            m[names["src"][b]] = src[gb]
            m[names["tgt"][b]] = tgt[gb]
        in_maps.append(m)

    res = run_bass_kernel_spmd(nc, in_maps, core_ids=list(range(NCORES)),
                               **spmd_kwargs)
    out = np.empty((B, W, W, 1), dtype=np.float32)
    for c in range(NCORES):
        for b in range(BLOC):
            out[c * BLOC + b, :, :, 0] = res.results[c][names["out"][b]].astype(
                np.float32)
    return out, res


def kernel(hidden, classifier_w, classifier_b, source_word_ids,
           target_word_ids, num_words):
    out, _ = _run(hidden, classifier_w, classifier_b, source_word_ids,
                  target_word_ids)
    return out


# revision 11
# speedup vs baseline: 1.8831x; 1.3090x over previous
"""Trainium2 Bass kernel for BinaryTokenClassificationModel (segment_reduce).

Reference semantics (B=16, L=2048, H=1024, W=1024):
    src = segment_mean(hidden, source_word_ids)   # [B,W,H]
    tgt = segment_mean(hidden, target_word_ids)   # [B,W,H]
    logits[b,s,t,0] = src[b,s]@w_s + tgt[b,t]@w_t + bias

Because the classifier is linear, the pooled [B,W,H] tensors are never
materialized:
    src_proj[b,s] = segment_mean_s( hidden[b,l] @ w_s )
so the per-token scalar dots are computed on the PE (hidden is
pre-transposed to [H,L] on the host so it streams through the PE as the
moving operand against a tiny stationary [128,2] weight tile), the
scalar dots are segment-reduced via one-hot matmuls on the PE (word =
128*q + r factorization), and the [W,W] output is an outer
broadcast-sum emitted in bf16 (upcast on the host).

The two batches per core are software-pipelined: batch 0's epilogue
(transposes, segment matmuls, broadcast, output adds) is emitted inside
batch 1's DMA-paced dot stream so the input DMA never stalls.

Sharding: data-parallel over batch — 2 examples per NeuronCore on 8
cores; classifier weights replicated.
"""

from contextlib import ExitStack

import ml_dtypes
import numpy as np

import concourse.mybir as mybir
import concourse.tile as tile
from concourse import bacc
from concourse.bass_utils import run_bass_kernel_spmd
from concourse.masks import make_identity

P = 128          # partitions
B = 16           # full batch
NCORES = 8
BLOC = B // NCORES   # batches per core = 2
L = 2048         # tokens
H = 1024         # hidden
W = 1024         # words
Q = W // P       # 8 word chunks
NI = L // P      # 16 token tiles per batch (token l = p*NI + i)
NH = H // P      # 8 hidden chunks
NCH = 4          # 512-token psum chunks for the dot matmuls
CHW = L // NCH   # 512

F32 = mybir.dt.float32
BF16 = mybir.dt.bfloat16
I32 = mybir.dt.int32

_CACHE = {}


def _build_module():
    nc = bacc.Bacc(None, target_bir_lowering=False, debug=False)
    names = {}
    with tile.TileContext(nc) as tc, ExitStack() as ctx:
        dram = ctx.enter_context(tc.tile_pool(name="dram", bufs=1, space="DRAM"))
        sb_c = ctx.enter_context(tc.tile_pool(name="const", bufs=1))
        sb_h = ctx.enter_context(tc.tile_pool(name="hid", bufs=5))
        sb_s = ctx.enter_context(tc.tile_pool(name="small", bufs=2))
        sb_o = ctx.enter_context(tc.tile_pool(name="outp", bufs=6))
        ps_d = ctx.enter_context(tc.tile_pool(name="psdot", bufs=1, space="PSUM"))
        ps = ctx.enter_context(tc.tile_pool(name="psum", bufs=1, space="PSUM"))

        hidT_d = [dram.tile([H, L], BF16, kind="ExternalInput", name=f"hidT{b}")
                  for b in range(BLOC)]
        src_d = [dram.tile([L], I32, kind="ExternalInput", name=f"srcids{b}")
                 for b in range(BLOC)]
        tgt_d = [dram.tile([L], I32, kind="ExternalInput", name=f"tgtids{b}")
                 for b in range(BLOC)]
        w_d = dram.tile([H, 2], BF16, kind="ExternalInput")   # [:,0]=w_s [:,1]=w_t
        b_d = dram.tile([P, 1], F32, kind="ExternalInput")
        out_d = [dram.tile([W, W], BF16, kind="ExternalOutput", name=f"logits{b}")
                 for b in range(BLOC)]

        names["hidT"] = [t.name for t in hidT_d]
        names["src"] = [t.name for t in src_d]
        names["tgt"] = [t.name for t in tgt_d]
        names["w"] = w_d.name
        names["b"] = b_d.name
        names["out"] = [t.name for t in out_d]

        # ---- constants ----
        # iota_r16[p, i, r] = r ; iota_q16[p, i, q] = q  (bf16: ints < 256 exact)
        iota_r16 = sb_c.tile([P, NI, P], BF16, tag="ior")
        nc.gpsimd.iota(iota_r16[:], pattern=[[0, NI], [1, P]], base=0,
                       channel_multiplier=0, allow_small_or_imprecise_dtypes=True)
        iota_q16 = sb_c.tile([P, NI, Q], BF16, tag="ioq")
        nc.gpsimd.iota(iota_q16[:], pattern=[[0, NI], [1, Q]], base=0,
                       channel_multiplier=0, allow_small_or_imprecise_dtypes=True)
        ident = sb_c.tile([P, P], BF16, tag="id")
        make_identity(nc, ident[:])
        ones = sb_c.tile([P, P], BF16, tag="ones")
        nc.vector.memset(ones[:], 1.0)
        ident2 = sb_c.tile([2, 2], F32, tag="id2")
        make_identity(nc, ident2[:])

        w2 = sb_c.tile([P, NH, 2], BF16, tag="w2")
        nc.scalar.dma_start(out=w2[:], in_=w_d[:].rearrange("(hq hp) s -> hp hq s",
                                                            hp=P))
        b_bc = sb_c.tile([P, 1], F32, tag="bb")
        nc.scalar.dma_start(out=b_bc[:], in_=b_d[:])

        # ---------------- stage builders ----------------

        def build_onehots(b):
            """ids -> (q, r) one-hots; ids math on Pool, is_eq split DVE/Pool."""
            oralls = {}
            mdoq = {}
            for side, ids_dram, eng in (("s", src_d[b], nc.vector),
                                        ("t", tgt_d[b], nc.vector)):
                ids_t = sb_s.tile([P, NI], I32, tag=f"ids{side}")
                nc.sync.dma_start(out=ids_t[:],
                                  in_=ids_dram[:].rearrange("(p i) -> p i", p=P))
                q_i = sb_s.tile([P, NI], I32, tag=f"qi{side}")
                r_i = sb_s.tile([P, NI], I32, tag=f"ri{side}")
                nc.vector.tensor_scalar(out=q_i[:], in0=ids_t[:], scalar1=7,
                                        scalar2=None,
                                        op0=mybir.AluOpType.logical_shift_right)
                nc.vector.tensor_scalar(out=r_i[:], in0=ids_t[:], scalar1=127,
                                        scalar2=None,
                                        op0=mybir.AluOpType.bitwise_and)
                qf = sb_s.tile([P, NI], BF16, tag=f"qf{side}")
                rf = sb_s.tile([P, NI], BF16, tag=f"rf{side}")
                nc.vector.tensor_copy(out=qf[:], in_=q_i[:])
                nc.vector.tensor_copy(out=rf[:], in_=r_i[:])
                oralls[side] = sb_s.tile([P, NI, P], BF16, tag=f"orall{side}",
                                         name=f"orall{side}")
                eng.tensor_tensor(
                    out=oralls[side][:], in0=iota_r16[:],
                    in1=rf[:].to_broadcast([P, NI, P]),
                    op=mybir.AluOpType.is_equal)
                mdoq[side] = sb_s.tile([P, NI, 2 * Q], BF16, tag=f"mdoq{side}",
                                       name=f"mdoq{side}")
                eng.tensor_tensor(
                    out=mdoq[side][:, :, Q:2 * Q], in0=iota_q16[:],
                    in1=qf[:].to_broadcast([P, NI, Q]),
                    op=mybir.AluOpType.is_equal)
            return oralls, mdoq

        def dots_chunk(b, pdots, hq):
            """One [128, L] h-chunk of hidden^T streamed through the PE."""
            ht = sb_h.tile([P, L], BF16, tag="ht")
            nc.sync.dma_start(out=ht[:], in_=hidT_d[b][hq * P:(hq + 1) * P, :])
            for n in range(NCH):
                nc.tensor.matmul(out=pdots[n][:], lhsT=w2[:, hq, :],
                                 rhs=ht[:, n * CHW:(n + 1) * CHW],
                                 start=(hq == 0), stop=(hq == NH - 1))

        def dance(b, pdots):
            """psum dot chunks -> token-partition layout psT[p, i, side]."""
            dots_sb = sb_s.tile([2, P, NI], F32, tag="dots", name="dots")
            for n in range(NCH):
                dst = dots_sb[:, n * (P // NCH):(n + 1) * (P // NCH), :]
                if n % 2 == 0:
                    nc.scalar.copy(out=dst, in_=pdots[n][:])
                else:
                    nc.vector.tensor_copy(out=dst, in_=pdots[n][:])
            psT = ps.tile([P, NI, 2], F32, space="PSUM", tag="psT", name="psT",
                          bufs=1)
            for i in range(NI):
                nc.tensor.transpose(out=psT[:, i, :], in_=dots_sb[:, :, i],
                                    identity=ident2[:])
            return psT

        def seg_reduce(b, psT, oralls, mdoq):
            """md = onehot_q * dots; segment sums+counts via PE matmuls."""
            segP = ps.tile([P, 2, 2 * Q], F32, space="PSUM", tag="segP",
                           name="segP", bufs=1)
            for sidx, side in enumerate(("s", "t")):
                nc.vector.tensor_tensor(
                    out=mdoq[side][:, :, 0:Q], in0=mdoq[side][:, :, Q:2 * Q],
                    in1=psT[:, :, sidx:sidx + 1].to_broadcast([P, NI, Q]),
                    op=mybir.AluOpType.mult)
                for i in range(NI):
                    nc.tensor.matmul(out=segP[:, sidx, :],
                                     lhsT=oralls[side][:, i, :],
                                     rhs=mdoq[side][:, i, :],
                                     start=(i == 0), stop=(i == NI - 1))
            return segP

        def epilogue(b, segP):
            """divide sums by counts; build [P, W] broadcast of tgt proj."""
            proj = {}
            for sidx, side in enumerate(("s", "t")):
                cnt = sb_s.tile([P, Q], F32, tag="cnt")
                nc.vector.tensor_scalar(out=cnt[:], in0=segP[:, sidx, Q:2 * Q],
                                        scalar1=1.0, scalar2=None,
                                        op0=mybir.AluOpType.max)
                rec = sb_s.tile([P, Q], F32, tag="rec")
                nc.vector.reciprocal(out=rec[:], in_=cnt[:])
                proj[side] = sb_s.tile([P, Q], F32, tag=f"proj{side}",
                                       name=f"proj{side}")
                nc.vector.tensor_tensor(out=proj[side][:],
                                        in0=segP[:, sidx, 0:Q],
                                        in1=rec[:], op=mybir.AluOpType.mult)

            proj_sb = sb_s.tile([P, Q], F32, tag="projsb")
            nc.vector.tensor_scalar(out=proj_sb[:], in0=proj["s"][:],
                                    scalar1=b_bc[:, 0:1], scalar2=None,
                                    op0=mybir.AluOpType.add)

            msel = sb_s.tile([P, W], BF16, tag="msel")
            for qb in range(Q):
                nc.vector.tensor_scalar(
                    out=msel[:, qb * P:(qb + 1) * P], in0=ident[:],
                    scalar1=proj["t"][:, qb:qb + 1], scalar2=None,
                    op0=mybir.AluOpType.mult)
            bc_sb = sb_s.tile([P, W], BF16, tag="bcsb")
            for half in range(2):
                bc_ps = ps.tile([P, W // 2], F32, space="PSUM", tag="bc",
                                bufs=2)
                nc.tensor.matmul(out=bc_ps[:], lhsT=ones[:],
                                 rhs=msel[:, half * (W // 2):(half + 1) * (W // 2)],
                                 start=True, stop=True)
                nc.scalar.copy(out=bc_sb[:, half * (W // 2):(half + 1) * (W // 2)],
                               in_=bc_ps[:])
            return proj_sb, bc_sb

        def outputs(b, proj_sb, bc_sb):
            """out[j*128+p, t] = proj_s[p, j] + tp[t]; adds on DVE,
            output DMA on the scalar queue (separate ring from loads)."""
            out_ap = out_d[b][:].rearrange("(j p) t -> p j t", p=P)
            for j in range(Q):
                ot = sb_o.tile([P, W], BF16, tag="ot")
                nc.vector.tensor_scalar(
                    out=ot[:], in0=bc_sb[:], scalar1=proj_sb[:, j:j + 1],
                    scalar2=None, op0=mybir.AluOpType.add)
                nc.scalar.dma_start(out=out_ap[:, j, :], in_=ot[:])

        # ---------------- pipelined emission ----------------
        oh0 = build_onehots(0)
        oh1 = build_onehots(1)

        pd0 = [ps_d.tile([2, P // NCH, NI], F32, space="PSUM", tag=f"pd{n}",
                         name=f"pd{n}") for n in range(NCH)]
        for hq in range(NH):
            dots_chunk(0, pd0, hq)
        psT0 = dance(0, pd0)

        # batch 1 dots start immediately (psum chunks freed by the dance
        # copies); batch 0's epilogue PE work rides in the DMA-paced gaps.
        pd1 = [ps_d.tile([2, P // NCH, NI], F32, space="PSUM", tag=f"pd{n}",
                         name=f"pd{n}") for n in range(NCH)]
        dots_chunk(1, pd1, 0)

        segP0 = seg_reduce(0, psT0, *oh0)
        proj0, bc0 = epilogue(0, segP0)

        for hq in range(1, NH):
            dots_chunk(1, pd1, hq)

        outputs(0, proj0, bc0)

        psT1 = dance(1, pd1)
        segP1 = seg_reduce(1, psT1, *oh1)
        proj1, bc1 = epilogue(1, segP1)
        outputs(1, proj1, bc1)

    nc.compile()
    return nc, names


def _get_module():
    if "mod" not in _CACHE:
        _CACHE["mod"] = _build_module()
    return _CACHE["mod"]


def _run(hidden, classifier_w, classifier_b, source_word_ids, target_word_ids,
         **spmd_kwargs):
    nc, names = _get_module()
    bf16 = ml_dtypes.bfloat16
    # [B, H, L] contiguous so each [128, L] h-chunk is a dense DMA
    hidT = np.ascontiguousarray(
        np.asarray(hidden).astype(bf16).transpose(0, 2, 1))
    w = np.asarray(classifier_w, dtype=np.float32).reshape(2 * H)
    w2 = np.ascontiguousarray(np.stack([w[:H], w[H:]], axis=-1).astype(bf16))
    bias = np.ascontiguousarray(
        np.broadcast_to(np.asarray(classifier_b, dtype=np.float32)
                        .reshape(1, 1), (P, 1)))
    src = np.ascontiguousarray(source_word_ids, dtype=np.int32)
    tgt = np.ascontiguousarray(target_word_ids, dtype=np.int32)

    in_maps = []
    for c in range(NCORES):
        m = {names["w"]: w2, names["b"]: bias}
        for b in range(BLOC):
            gb = c * BLOC + b
            m[names["hidT"][b]] = hidT[gb]
            m[names["src"][b]] = src[gb]
            m[names["tgt"][b]] = tgt[gb]
        in_maps.append(m)

    res = run_bass_kernel_spmd(nc, in_maps, core_ids=list(range(NCORES)),
                               **spmd_kwargs)
    out = np.empty((B, W, W, 1), dtype=np.float32)
    for c in range(NCORES):
        for b in range(BLOC):
            out[c * BLOC + b, :, :, 0] = res.results[c][names["out"][b]].astype(
                np.float32)
    return out, res


def kernel(hidden, classifier_w, classifier_b, source_word_ids,
           target_word_ids, num_words):
    out, _ = _run(hidden, classifier_w, classifier_b, source_word_ids,
                  target_word_ids)
    return out
